# revision 49
# baseline (speedup 1.0000x reference)
"""Trainium2 Bass kernel for nn_Net2_54494545051831 (LocallyConnected2d(7x7)
-> bias -> ReLU -> Linear(28392 -> 10)), on 8 NeuronCores.

Distribution: by output location. Each core owns 3 full output rows
(h = 3c .. 3c+2) plus a 6-7 wide piece of rows 24/25 -> 84/85 locations.
Weights / bias / lw are sharded by location (nothing replicated); each core
computes a partial [10, B] of the final linear layer, summed on host.

Per-core compute ("band" layout): for each owned output row, x is reordered
host-side so the contraction rows of location (h, w) sit at band partitions
[22w, 22w+153): band row 22*w' + j = x[:, j//7, h + j%7, w'] for j < 21,
1.0 at j == 21 (bias folds into the weights), 0 above.

Locations are processed in GROUPS of 3 (42*3 = 126 output channels + 2 zero
columns = full 128-wide stationary operand). A group's 3 windows span
[22w0, 22w0+197) -> 2-3 aligned 128-row band tiles; one full-width matmul
per (group, tile) with zero weights on rows outside each location's window.
ReLU alternates between Vector and Scalar engines; the linear layer
contracts each group's relu'd [128, 512] tile with a [128, 10] per-group lw
slice, accumulated in 4 PSUM column-tile slices (concurrent col-tiled
matmuls) that are summed on host. bf16 matmuls with fp32 accumulation.

Schedule: PE warm-up matmuls keep HAM at K=8/8 through the DMA prologue;
band/weights stream through double-buffered row-granular tile pools whose
WAR dependencies enforce arrival order; a dummy pool tile read by a late
warm-up matmul delays row 1's transfer so row 0 gets exclusive bandwidth.
"""
import numpy as np
import ml_dtypes

import concourse.mybir as mybir
import concourse.tile as tile
from concourse import bacc
from concourse.bass_utils import run_bass_kernel_spmd

BF16 = mybir.dt.bfloat16
F32 = mybir.dt.float32
RELU = mybir.ActivationFunctionType.Relu

B = 1024
IC, OC, NCLS = 3, 42, 10
KH = KW = 7
OH = OW = 26
NCORES = 8
N_ROWS = 4           # canonical band rows per core (3 full + 1 piece)
STRIDE = 22          # band rows per w'-block: 21 data + 1 ones(bias) row
WINLEN = 6 * STRIDE + 21   # partition span of one location window (153)
TPR = 6              # band tiles per canonical row (704 rows -> 6 tiles)
NB = 2               # two N-chunks of 512
NCHUNK = 512
N_WARM = 8           # PE warm-up matmuls during the DMA prologue

# Groups of consecutive locations within a canonical row: (w0, len)
GROUPS_FULL = [(0, 3), (3, 3), (6, 3), (9, 3), (12, 3), (15, 3), (18, 3),
               (21, 3), (24, 2)]
GROUPS_ROW3 = [(0, 3), (3, 3), (6, 2)]


def _group_tiles(w0, L):
    ta = (STRIDE * w0) // 128
    tb = (STRIDE * (w0 + L - 1) + WINLEN - 1) // 128
    return list(range(ta, tb + 1))


def _groups():
    """[(row, w0, L, [tiles], chunk0)] — chunk0 = first wt chunk index."""
    out = []
    ck = 0
    for r in range(N_ROWS):
        for w0, L in (GROUPS_FULL if r < 3 else GROUPS_ROW3):
            ts = _group_tiles(w0, L)
            out.append((r, w0, L, ts, ck))
            ck += len(ts)
    return out, ck

GROUPS, N_CHUNK_TOT = _groups()
NG = len(GROUPS)
# band tiles actually used per canonical row (row 3 only needs tiles 0-2)
ROW_TILES = [TPR, TPR, TPR, max(t for (r, _, _, ts, _) in GROUPS if r == 3
                                for t in ts) + 1]

_cache = {}


def _build_program():
    if "nc" in _cache:
        return _cache["nc"]

    nc = bacc.Bacc("TRN2", target_bir_lowering=False, debug=False,
                   num_devices=NCORES)
    band_d = nc.dram_tensor("band", [N_ROWS * TPR, 128, B], BF16,
                            kind="ExternalInput").ap()
    wt_d = nc.dram_tensor("wt", [128, N_CHUNK_TOT * 128], BF16,
                          kind="ExternalInput").ap()
    lwp_d = nc.dram_tensor("lwp", [128, NG * NCLS], BF16,
                           kind="ExternalInput").ap()
    # 4 col-tile partial slices, summed on host
    part_d = nc.dram_tensor("part", [4, NCLS, NB * NCHUNK], F32,
                            kind="ExternalOutput").ap()

    with tile.TileContext(nc) as tc:
        with (
            tc.tile_pool(name="sb", bufs=1) as sb,
            tc.tile_pool(name="band_pool", bufs=2) as band_pool,
            tc.tile_pool(name="wt_pool", bufs=2) as wt_pool,
            tc.tile_pool(name="stk_pool", bufs=8) as stk_pool,
            tc.tile_pool(name="pp_pool", bufs=5, space="PSUM") as pp_pool,
            tc.tile_pool(name="lin_pool", bufs=1, space="PSUM") as lin_pool,
            tc.tile_pool(name="warm_pool", bufs=1, space="PSUM") as warm_pool,
        ):
            lwp_s = sb.tile([128, NG * NCLS], BF16)
            zz = sb.tile([128, NCHUNK], BF16)

            # Band/weights are double-buffered row-granular pool tiles: row
            # r+2's DMA carries a WAR dependency on row r's last reader, so
            # in-flight transfers never steal bandwidth from the rows the PE
            # needs now — no matter how the scheduler orders the triggers.
            # Row 0 itself is split in two pieces; "gate" matmuls (garbage
            # reads, PSUM-WAW-anchored into the warm-up chain or RAW-anchored
            # on a relu output) give each later transfer a dependency that
            # fires exactly when the bandwidth frees up.
            row_groups = [[g for g in GROUPS if g[0] == r]
                          for r in range(N_ROWS)]
            WTW = max(gs[-1][4] + len(gs[-1][3]) - gs[0][4]
                      for gs in row_groups)        # chunks per row (<= 22)

            nc.gpsimd.memset(zz, 0.0)
            dummy_b = band_pool.tile([128, TPR * B], BF16, tag="band")
            dummy_w = wt_pool.tile([128, WTW * 128], BF16, tag="wt")
            nc.gpsimd.memset(dummy_b[:, 0:128], 0.0)
            nc.gpsimd.memset(dummy_w[:, 0:128], 0.0)
            nc.gpsimd.dma_start(out=lwp_s, in_=lwp_d)

            # Row 0 piece 0: band tiles 0-2 + weight chunks of groups 0-2 —
            # the minimum to start computing, lands ~3µs earlier than the
            # whole row.
            R0_SPLIT = 3          # groups 0-2 / tiles 0-2 / chunks 0-6
            g0s = row_groups[0]
            r0_c1 = g0s[-1][4] + len(g0s[-1][3])
            r0_cm = g0s[R0_SPLIT][4]
            bt0 = band_pool.tile([128, TPR * B], BF16, tag="band")
            wt0 = wt_pool.tile([128, WTW * 128], BF16, tag="wt")
            nc.sync.dma_start(out=bt0[:, 0:3 * B],
                              in_=band_d[0:3].transpose([1, 0, 2]))
            nc.scalar.dma_start(out=wt0[:, 0:r0_cm * 128],
                                in_=wt_d[:, 0:r0_cm * 128])

            # PE warm-up (HAM to K=8/8). The last warm-up matmul reads the
            # not-yet-written piece-1 regions of bt0/wt0: piece 1's DMAs
            # inherit that WAR dependency, so they only start once the
            # warm-up chain (≈ piece 0's landing) has run.
            warm_ps = warm_pool.tile([128, NCHUNK], F32, name="warm_ps")
            for i in range(N_WARM - 1):
                nc.tensor.matmul(warm_ps, zz[:, 0:128], zz,
                                 start=True, stop=True)
            nc.tensor.matmul(warm_ps, wt0[:, r0_cm * 128:(r0_cm + 1) * 128],
                             bt0[:, 3 * B:3 * B + NCHUNK],
                             start=True, stop=True)

            # Row 0 piece 1
            nc.sync.dma_start(out=bt0[:, 3 * B:TPR * B],
                              in_=band_d[3:TPR].transpose([1, 0, 2]))
            nc.scalar.dma_start(out=wt0[:, r0_cm * 128:r0_c1 * 128],
                                in_=wt_d[:, r0_cm * 128:r0_c1 * 128])

            def start_row(r, split=1):
                gs = row_groups[r]
                nt = ROW_TILES[r]
                half = (nt + 1) // 2
                bt = band_pool.tile([128, TPR * B], BF16, tag="band")
                nc.sync.dma_start(
                    out=bt[:, 0:half * B],
                    in_=band_d[r * TPR:r * TPR + half].transpose([1, 0, 2]))
                nc.sync.dma_start(
                    out=bt[:, half * B:nt * B],
                    in_=band_d[r * TPR + half:r * TPR + nt]
                    .transpose([1, 0, 2]))
                c0 = gs[0][4]
                c1 = gs[-1][4] + len(gs[-1][3])
                wtt = wt_pool.tile([128, WTW * 128], BF16, tag="wt")
                cm = c0 + (c1 - c0) // split
                nc.scalar.dma_start(out=wtt[:, 0:(cm - c0) * 128],
                                    in_=wt_d[:, c0 * 128:cm * 128])
                if cm < c1:
                    nc.scalar.dma_start(out=wtt[:, (cm - c0) * 128:
                                               (c1 - c0) * 128],
                                        in_=wt_d[:, cm * 128:c1 * 128])
                return bt, wtt, c0

            # Linear layer: 4 PSUM column-tile slices per nb; groups are
            # assigned round-robin to col positions (0,32,64,96) and each
            # batch of 4 linear matmuls is emitted back-to-back so they run
            # concurrently in disjoint PE column groups.
            lin_ps = [lin_pool.tile([128, NCHUNK], F32, name=f"lin_ps{nb}")
                      for nb in range(NB)]
            # per (nb, pos): how many groups land there (for start/stop)
            npos = [[0] * 4 for _ in range(NB)]
            for k in range(NG):
                npos[0][k % 4] += 1
                npos[1][k % 4] += 1
            lin_done = [[0] * 4 for _ in range(NB)]

            def emit_lin(gi, nb, stk, k):
                pos = k % 4
                seen = lin_done[nb][pos]
                lin_done[nb][pos] += 1
                nc.tensor.matmul(
                    lin_ps[nb][32 * pos:32 * pos + NCLS, :],
                    lwp_s[:, gi * NCLS:(gi + 1) * NCLS],
                    stk,
                    start=(seen == 0), stop=(seen == npos[nb][pos] - 1),
                    tile_position=(0, 32 * pos),
                    skip_group_check=True,
                )

            relu_i = [0]

            def emit_relu(stk, pp, eng=None):
                k = relu_i[0] % 2
                relu_i[0] += 1
                if eng == "s" or (eng is None and k == 1):
                    nc.scalar.activation(stk, pp, RELU)
                else:
                    nc.vector.tensor_scalar_max(stk, pp, 0.0)

            pend = []
            lin_k = [0, 0]   # per-nb emitted-lin counter (drives col pos)

            def flush_lin(nmin):
                while len(pend) >= nmin:
                    batch = [pend.pop(0) for _ in range(min(4, len(pend)))]
                    for (gi, nb, stk) in batch:
                        emit_lin(gi, nb, stk, lin_k[nb])
                        lin_k[nb] += 1

            def do_group(bt_s, wt_s, cbase, g, nb):
                (gr, w0, L, ts, ck) = g
                gi = GROUPS.index(g)
                pp = pp_pool.tile([128, NCHUNK], F32, tag="pp")
                for ci, t in enumerate(ts):
                    cc = ck - cbase + ci
                    nc.tensor.matmul(
                        pp,
                        wt_s[:, cc * 128:(cc + 1) * 128],
                        bt_s[:, t * B + nb * NCHUNK:
                             t * B + nb * NCHUNK + NCHUNK],
                        start=(ci == 0), stop=(ci == len(ts) - 1),
                    )
                stk = stk_pool.tile([128, NCHUNK], BF16, tag="stk")
                emit_relu(stk, pp)
                pend.append((gi, nb, stk))
                flush_lin(6)
                return stk

            # Row 0 section 0 (groups 0-2, piece-0 data only)
            anchor = None
            for nb in range(NB):
                for g in g0s[0:R0_SPLIT]:
                    anchor = do_group(bt0, wt0, 0, g, nb)
            # Gate matmuls: RAW on a section-0 relu output anchors them ~60%
            # into row 0; they are the sole readers of the dummy pool tiles,
            # so row 1's band/wt DMAs (pool WAR) start exactly then.
            nc.tensor.matmul(warm_ps, dummy_b[:, 0:128], anchor,
                             start=True, stop=True)
            nc.tensor.matmul(warm_ps, dummy_w[:, 0:128], anchor,
                             start=True, stop=True)
            cur = (bt0, wt0, 0)
            nxt = start_row(1)
            # Row 0 section 1, then rows 1-3
            for nb in range(NB):
                for g in g0s[R0_SPLIT:]:
                    do_group(bt0, wt0, 0, g, nb)
            for r in range(1, N_ROWS):
                if r + 1 < N_ROWS:
                    cur, nxt = nxt, start_row(r + 1)
                else:
                    cur, nxt = nxt, None
                bt_s, wt_s, cbase = cur
                for nb in range(NB):
                    for g in row_groups[r]:
                        do_group(bt_s, wt_s, cbase, g, nb)
            flush_lin(1)
            out_s = sb.tile([106, NB * NCHUNK], F32)
            nc.vector.tensor_copy(out_s[:, 0:NCHUNK], lin_ps[0][0:106, :])
            nc.scalar.activation(out_s[:, NCHUNK:2 * NCHUNK],
                                 lin_ps[1][0:106, :],
                                 mybir.ActivationFunctionType.Copy)
            oeng = [nc.sync, nc.gpsimd, nc.scalar, nc.sync]
            for pos in range(4):
                oeng[pos].dma_start(
                    out=part_d[pos],
                    in_=out_s[32 * pos:32 * pos + NCLS, :])

    nc.compile()
    _cache["nc"] = nc
    return nc


def _core_slots(c):
    """Actual (h, w) per canonical slot for core c; None = pad."""
    slots = []
    for i in range(78):
        slots.append((3 * c + i // 26, i % 26))
    p0 = (52 * c) // 8
    p1 = (52 * (c + 1)) // 8
    ph, pw0 = 24 + p0 // 26, p0 % 26
    plen = p1 - p0
    for j in range(8):
        slots.append((ph, pw0 + j) if j < plen else None)
    return slots, (ph, pw0, plen)


def _prep_core(c, x, W, b, lw4):
    """Build band / wt / lwp arrays for core c."""
    slots, (ph, pw0, plen) = _core_slots(c)

    # bands ------------------------------------------------------------
    hs = [(3 * c, 0), (3 * c + 1, 0), (3 * c + 2, 0), (ph, pw0)]
    band = np.zeros((N_ROWS * TPR, 128, B), dtype=ml_dtypes.bfloat16)
    cj = np.arange(21) // 7          # channel per j
    kij = np.arange(21) % 7          # kernel-row per j
    for r, (h, shift) in enumerate(hs):
        nblocks = min(32, 32 - shift)
        wslice = np.arange(nblocks) + shift
        blk = x[:, cj[:, None], (h + kij)[:, None], wslice[None, :]]
        blk = blk.transpose(1, 2, 0)          # [21, nblocks, B]
        brow = np.zeros((TPR * 128, B), dtype=ml_dtypes.bfloat16)
        for bw in range(nblocks):
            brow[STRIDE * bw:STRIDE * bw + 21] = blk[:, bw]
            brow[STRIDE * bw + 21] = 1.0
        band[r * TPR:(r + 1) * TPR] = brow.reshape(TPR, 128, B)
    # wt ----------------------------------------------------------------
    wt = np.zeros((128, N_CHUNK_TOT * 128), dtype=ml_dtypes.bfloat16)
    for (r, w0, L, ts, ck) in GROUPS:
        for s in range(L):
            w_c = w0 + s
            sl = r * 26 + w_c if r < 3 else 78 + w_c
            hw = slots[sl]
            if hw is None:
                continue
            h, w = hw
            Wl = W[:, :, h, w, :]                 # [42, 3, 49]
            bl = b[:, h, w]                       # [42]
            for ci, t in enumerate(ts):
                col = (ck + ci) * 128 + 42 * s
                rel = 128 * t + np.arange(128) - STRIDE * w_c
                kj = rel // STRIDE
                jj = rel % STRIDE
                valid = (rel >= 0) & (rel < WINLEN) & (jj < 21)
                vals = np.zeros((128, OC), dtype=np.float32)
                vj, vk = jj[valid], kj[valid]
                vals[valid] = Wl[:, vj // 7, (vj % 7) * 7 + vk].T
                bias_row = (rel >= 0) & (rel < WINLEN) & (jj == 21) & (kj == 0)
                if bias_row.any():
                    vals[bias_row] = bl
                wt[:, col:col + OC] = vals.astype(ml_dtypes.bfloat16)
    # lwp ---------------------------------------------------------------
    lwp = np.zeros((128, NG * NCLS), dtype=ml_dtypes.bfloat16)
    for gi, (r, w0, L, ts, ck) in enumerate(GROUPS):
        for s in range(L):
            w_c = w0 + s
            sl = r * 26 + w_c if r < 3 else 78 + w_c
            if slots[sl] is None:
                continue
            h, w = slots[sl]
            lwp[42 * s:42 * s + OC, gi * NCLS:(gi + 1) * NCLS] = (
                lw4[:, :, h, w].T.astype(ml_dtypes.bfloat16)
            )
    return {"band": band, "wt": wt, "lwp": lwp}


def _run(x, W, b, lw, lb, trace=False):
    nc = _build_program()
    x = np.ascontiguousarray(np.asarray(x, dtype=np.float32))
    W = np.asarray(W, dtype=np.float32)
    b = np.asarray(b, dtype=np.float32)
    lw = np.asarray(lw, dtype=np.float32)
    lb = np.asarray(lb, dtype=np.float32)
    lw4 = lw.reshape(NCLS, OC, OH, OW)
    in_maps = [_prep_core(c, x, W, b, lw4) for c in range(NCORES)]
    res = run_bass_kernel_spmd(
        nc, in_maps, list(range(NCORES)), trace=trace,
    )
    part = np.zeros((4, NCLS, NB * NCHUNK), dtype=np.float32)
    for c in range(NCORES):
        part += res.results[c]["part"]
    out10 = part.sum(axis=0)
    out = out10.T + lb[None, :]
    return out.astype(np.float32), res


def kernel(**inputs):
    out, _ = _run(inputs["x"], inputs["W"], inputs["b"], inputs["lw"],
                  inputs["lb"])
    return out


# revision 50
# speedup vs baseline: 1.0014x; 1.0014x over previous
"""Trainium2 Bass kernel for nn_Net2_54494545051831 (LocallyConnected2d(7x7)
-> bias -> ReLU -> Linear(28392 -> 10)), on 8 NeuronCores.

Distribution: by output location. Each core owns 3 full output rows
(h = 3c .. 3c+2) plus a 6-7 wide piece of rows 24/25 -> 84/85 locations.
Weights / bias / lw are sharded by location (nothing replicated); each core
computes a partial [10, B] of the final linear layer, summed on host.

Per-core compute ("band" layout): for each owned output row, x is reordered
host-side so the contraction rows of location (h, w) sit at band partitions
[22w, 22w+153): band row 22*w' + j = x[:, j//7, h + j%7, w'] for j < 21,
1.0 at j == 21 (bias folds into the weights), 0 above.

Locations are processed in GROUPS of 3 (42*3 = 126 output channels + 2 zero
columns = full 128-wide stationary operand). A group's 3 windows span
[22w0, 22w0+197) -> 2-3 aligned 128-row band tiles; one full-width matmul
per (group, tile) with zero weights on rows outside each location's window.
ReLU alternates between Vector and Scalar engines; the linear layer
contracts each group's relu'd [128, 512] tile with a [128, 10] per-group lw
slice, accumulated in 4 PSUM column-tile slices (concurrent col-tiled
matmuls) that are summed on host. bf16 matmuls with fp32 accumulation.

Schedule: PE warm-up matmuls keep HAM at K=8/8 through the DMA prologue;
band/weights stream through double-buffered row-granular tile pools whose
WAR dependencies enforce arrival order; a dummy pool tile read by a late
warm-up matmul delays row 1's transfer so row 0 gets exclusive bandwidth.
"""
import numpy as np
import ml_dtypes

import concourse.mybir as mybir
import concourse.tile as tile
from concourse import bacc
from concourse.bass_utils import run_bass_kernel_spmd

BF16 = mybir.dt.bfloat16
F32 = mybir.dt.float32
RELU = mybir.ActivationFunctionType.Relu

B = 1024
IC, OC, NCLS = 3, 42, 10
KH = KW = 7
OH = OW = 26
NCORES = 8
N_ROWS = 4           # canonical band rows per core (3 full + 1 piece)
STRIDE = 22          # band rows per w'-block: 21 data + 1 ones(bias) row
WINLEN = 6 * STRIDE + 21   # partition span of one location window (153)
TPR = 6              # band tiles per canonical row (704 rows -> 6 tiles)
NB = 2               # two N-chunks of 512
NCHUNK = 512
N_WARM = 12          # PE warm-up matmuls during the DMA prologue

# Groups of consecutive locations within a canonical row: (w0, len)
GROUPS_FULL = [(0, 3), (3, 3), (6, 3), (9, 3), (12, 3), (15, 3), (18, 3),
               (21, 3), (24, 2)]
GROUPS_ROW3 = [(0, 3), (3, 3), (6, 2)]


def _group_tiles(w0, L):
    ta = (STRIDE * w0) // 128
    tb = (STRIDE * (w0 + L - 1) + WINLEN - 1) // 128
    return list(range(ta, tb + 1))


def _groups():
    """[(row, w0, L, [tiles], chunk0)] — chunk0 = first wt chunk index."""
    out = []
    ck = 0
    for r in range(N_ROWS):
        for w0, L in (GROUPS_FULL if r < 3 else GROUPS_ROW3):
            ts = _group_tiles(w0, L)
            out.append((r, w0, L, ts, ck))
            ck += len(ts)
    return out, ck

GROUPS, N_CHUNK_TOT = _groups()
NG = len(GROUPS)
# band tiles actually used per canonical row (row 3 only needs tiles 0-2)
ROW_TILES = [TPR, TPR, TPR, max(t for (r, _, _, ts, _) in GROUPS if r == 3
                                for t in ts) + 1]

_cache = {}


def _build_program():
    if "nc" in _cache:
        return _cache["nc"]

    nc = bacc.Bacc("TRN2", target_bir_lowering=False, debug=False,
                   num_devices=NCORES)
    band_d = nc.dram_tensor("band", [N_ROWS * TPR, 128, B], BF16,
                            kind="ExternalInput").ap()
    wt_d = nc.dram_tensor("wt", [128, N_CHUNK_TOT * 128], BF16,
                          kind="ExternalInput").ap()
    lwp_d = nc.dram_tensor("lwp", [128, NG * NCLS], BF16,
                           kind="ExternalInput").ap()
    # 4 col-tile partial slices, summed on host
    part_d = nc.dram_tensor("part", [4, NCLS, NB * NCHUNK], F32,
                            kind="ExternalOutput").ap()

    with tile.TileContext(nc) as tc:
        with (
            tc.tile_pool(name="sb", bufs=1) as sb,
            tc.tile_pool(name="band_pool", bufs=2) as band_pool,
            tc.tile_pool(name="wt_pool", bufs=2) as wt_pool,
            tc.tile_pool(name="stk_pool", bufs=8) as stk_pool,
            tc.tile_pool(name="pp_pool", bufs=5, space="PSUM") as pp_pool,
            tc.tile_pool(name="lin_pool", bufs=1, space="PSUM") as lin_pool,
            tc.tile_pool(name="warm_pool", bufs=1, space="PSUM") as warm_pool,
        ):
            lwp_s = sb.tile([128, NG * NCLS], BF16)
            zz = sb.tile([128, NCHUNK], BF16)

            # Band/weights are double-buffered row-granular pool tiles: row
            # r+2's DMA carries a WAR dependency on row r's last reader, so
            # in-flight transfers never steal bandwidth from the rows the PE
            # needs now — no matter how the scheduler orders the triggers.
            # Row 0 itself is split in two pieces; "gate" matmuls (garbage
            # reads, PSUM-WAW-anchored into the warm-up chain or RAW-anchored
            # on a relu output) give each later transfer a dependency that
            # fires exactly when the bandwidth frees up.
            row_groups = [[g for g in GROUPS if g[0] == r]
                          for r in range(N_ROWS)]
            WTW = max(gs[-1][4] + len(gs[-1][3]) - gs[0][4]
                      for gs in row_groups)        # chunks per row (<= 22)

            nc.gpsimd.memset(zz, 0.0)
            dummy_b = band_pool.tile([128, TPR * B], BF16, tag="band")
            dummy_w = wt_pool.tile([128, WTW * 128], BF16, tag="wt")
            nc.gpsimd.memset(dummy_b[:, 0:128], 0.0)
            nc.gpsimd.memset(dummy_w[:, 0:128], 0.0)
            nc.gpsimd.dma_start(out=lwp_s, in_=lwp_d)

            # Row 0 piece 0: band tiles 0-2 + weight chunks of groups 0-2 —
            # the minimum to start computing, lands ~3µs earlier than the
            # whole row.
            R0_SPLIT = 3          # groups 0-2 / tiles 0-2 / chunks 0-6
            g0s = row_groups[0]
            r0_c1 = g0s[-1][4] + len(g0s[-1][3])
            r0_cm = g0s[R0_SPLIT][4]
            bt0 = band_pool.tile([128, TPR * B], BF16, tag="band")
            wt0 = wt_pool.tile([128, WTW * 128], BF16, tag="wt")
            nc.sync.dma_start(out=bt0[:, 0:3 * B],
                              in_=band_d[0:3].transpose([1, 0, 2]))
            nc.scalar.dma_start(out=wt0[:, 0:r0_cm * 128],
                                in_=wt_d[:, 0:r0_cm * 128])

            # PE warm-up (HAM to K=8/8). The last warm-up matmul reads the
            # not-yet-written piece-1 regions of bt0/wt0: piece 1's DMAs
            # inherit that WAR dependency, so they only start once the
            # warm-up chain (≈ piece 0's landing) has run.
            warm_ps = warm_pool.tile([128, NCHUNK], F32, name="warm_ps")
            for i in range(N_WARM - 1):
                nc.tensor.matmul(warm_ps, zz[:, 0:128], zz,
                                 start=True, stop=True)
            nc.tensor.matmul(warm_ps, wt0[:, r0_cm * 128:(r0_cm + 1) * 128],
                             bt0[:, 3 * B:3 * B + NCHUNK],
                             start=True, stop=True)

            # Row 0 piece 1
            nc.sync.dma_start(out=bt0[:, 3 * B:TPR * B],
                              in_=band_d[3:TPR].transpose([1, 0, 2]))
            nc.scalar.dma_start(out=wt0[:, r0_cm * 128:r0_c1 * 128],
                                in_=wt_d[:, r0_cm * 128:r0_c1 * 128])

            def start_row(r, split=1):
                gs = row_groups[r]
                nt = ROW_TILES[r]
                half = (nt + 1) // 2
                bt = band_pool.tile([128, TPR * B], BF16, tag="band")
                nc.sync.dma_start(
                    out=bt[:, 0:half * B],
                    in_=band_d[r * TPR:r * TPR + half].transpose([1, 0, 2]))
                nc.sync.dma_start(
                    out=bt[:, half * B:nt * B],
                    in_=band_d[r * TPR + half:r * TPR + nt]
                    .transpose([1, 0, 2]))
                c0 = gs[0][4]
                c1 = gs[-1][4] + len(gs[-1][3])
                wtt = wt_pool.tile([128, WTW * 128], BF16, tag="wt")
                cm = c0 + (c1 - c0) // split
                nc.scalar.dma_start(out=wtt[:, 0:(cm - c0) * 128],
                                    in_=wt_d[:, c0 * 128:cm * 128])
                if cm < c1:
                    nc.scalar.dma_start(out=wtt[:, (cm - c0) * 128:
                                               (c1 - c0) * 128],
                                        in_=wt_d[:, cm * 128:c1 * 128])
                return bt, wtt, c0

            # Linear layer: 4 PSUM column-tile slices per nb; groups are
            # assigned round-robin to col positions (0,32,64,96) and each
            # batch of 4 linear matmuls is emitted back-to-back so they run
            # concurrently in disjoint PE column groups.
            lin_ps = [lin_pool.tile([128, NCHUNK], F32, name=f"lin_ps{nb}")
                      for nb in range(NB)]
            # per (nb, pos): how many groups land there (for start/stop)
            npos = [[0] * 4 for _ in range(NB)]
            for k in range(NG):
                npos[0][k % 4] += 1
                npos[1][k % 4] += 1
            lin_done = [[0] * 4 for _ in range(NB)]

            def emit_lin(gi, nb, stk, k):
                pos = k % 4
                seen = lin_done[nb][pos]
                lin_done[nb][pos] += 1
                nc.tensor.matmul(
                    lin_ps[nb][32 * pos:32 * pos + NCLS, :],
                    lwp_s[:, gi * NCLS:(gi + 1) * NCLS],
                    stk,
                    start=(seen == 0), stop=(seen == npos[nb][pos] - 1),
                    tile_position=(0, 32 * pos),
                    skip_group_check=True,
                )

            relu_i = [0]

            def emit_relu(stk, pp, eng=None):
                k = relu_i[0] % 2
                relu_i[0] += 1
                if eng == "s" or (eng is None and k == 1):
                    nc.scalar.activation(stk, pp, RELU)
                else:
                    nc.vector.tensor_scalar_max(stk, pp, 0.0)

            pend = []
            lin_k = [0, 0]   # per-nb emitted-lin counter (drives col pos)

            def flush_lin(nmin):
                while len(pend) >= nmin:
                    batch = [pend.pop(0) for _ in range(min(4, len(pend)))]
                    for (gi, nb, stk) in batch:
                        emit_lin(gi, nb, stk, lin_k[nb])
                        lin_k[nb] += 1

            def do_group(bt_s, wt_s, cbase, g, nb):
                (gr, w0, L, ts, ck) = g
                gi = GROUPS.index(g)
                pp = pp_pool.tile([128, NCHUNK], F32, tag="pp")
                for ci, t in enumerate(ts):
                    cc = ck - cbase + ci
                    nc.tensor.matmul(
                        pp,
                        wt_s[:, cc * 128:(cc + 1) * 128],
                        bt_s[:, t * B + nb * NCHUNK:
                             t * B + nb * NCHUNK + NCHUNK],
                        start=(ci == 0), stop=(ci == len(ts) - 1),
                    )
                stk = stk_pool.tile([128, NCHUNK], BF16, tag="stk")
                emit_relu(stk, pp)
                pend.append((gi, nb, stk))
                flush_lin(6)
                return stk

            # Row 0 section 0 (groups 0-2, piece-0 data only)
            anchor = None
            for nb in range(NB):
                for g in g0s[0:R0_SPLIT]:
                    anchor = do_group(bt0, wt0, 0, g, nb)
            # Gate matmuls: RAW on a section-0 relu output anchors them ~60%
            # into row 0; they are the sole readers of the dummy pool tiles,
            # so row 1's band/wt DMAs (pool WAR) start exactly then.
            nc.tensor.matmul(warm_ps, dummy_b[:, 0:128], anchor,
                             start=True, stop=True)
            nc.tensor.matmul(warm_ps, dummy_w[:, 0:128], anchor,
                             start=True, stop=True)
            cur = (bt0, wt0, 0)
            nxt = start_row(1)
            # Row 0 section 1, then rows 1-3
            for nb in range(NB):
                for g in g0s[R0_SPLIT:]:
                    do_group(bt0, wt0, 0, g, nb)
            for r in range(1, N_ROWS):
                if r + 1 < N_ROWS:
                    cur, nxt = nxt, start_row(r + 1)
                else:
                    cur, nxt = nxt, None
                bt_s, wt_s, cbase = cur
                for nb in range(NB):
                    for g in row_groups[r]:
                        do_group(bt_s, wt_s, cbase, g, nb)
            flush_lin(1)
            out_s = sb.tile([106, NB * NCHUNK], F32)
            nc.vector.tensor_copy(out_s[:, 0:NCHUNK], lin_ps[0][0:106, :])
            nc.scalar.activation(out_s[:, NCHUNK:2 * NCHUNK],
                                 lin_ps[1][0:106, :],
                                 mybir.ActivationFunctionType.Copy)
            oeng = [nc.sync, nc.gpsimd, nc.scalar, nc.sync]
            for pos in range(4):
                oeng[pos].dma_start(
                    out=part_d[pos],
                    in_=out_s[32 * pos:32 * pos + NCLS, :])

    nc.compile()
    _cache["nc"] = nc
    return nc


def _core_slots(c):
    """Actual (h, w) per canonical slot for core c; None = pad."""
    slots = []
    for i in range(78):
        slots.append((3 * c + i // 26, i % 26))
    p0 = (52 * c) // 8
    p1 = (52 * (c + 1)) // 8
    ph, pw0 = 24 + p0 // 26, p0 % 26
    plen = p1 - p0
    for j in range(8):
        slots.append((ph, pw0 + j) if j < plen else None)
    return slots, (ph, pw0, plen)


def _prep_core(c, x, W, b, lw4):
    """Build band / wt / lwp arrays for core c."""
    slots, (ph, pw0, plen) = _core_slots(c)

    # bands ------------------------------------------------------------
    hs = [(3 * c, 0), (3 * c + 1, 0), (3 * c + 2, 0), (ph, pw0)]
    band = np.zeros((N_ROWS * TPR, 128, B), dtype=ml_dtypes.bfloat16)
    cj = np.arange(21) // 7          # channel per j
    kij = np.arange(21) % 7          # kernel-row per j
    for r, (h, shift) in enumerate(hs):
        nblocks = min(32, 32 - shift)
        wslice = np.arange(nblocks) + shift
        blk = x[:, cj[:, None], (h + kij)[:, None], wslice[None, :]]
        blk = blk.transpose(1, 2, 0)          # [21, nblocks, B]
        brow = np.zeros((TPR * 128, B), dtype=ml_dtypes.bfloat16)
        for bw in range(nblocks):
            brow[STRIDE * bw:STRIDE * bw + 21] = blk[:, bw]
            brow[STRIDE * bw + 21] = 1.0
        band[r * TPR:(r + 1) * TPR] = brow.reshape(TPR, 128, B)
    # wt ----------------------------------------------------------------
    wt = np.zeros((128, N_CHUNK_TOT * 128), dtype=ml_dtypes.bfloat16)
    for (r, w0, L, ts, ck) in GROUPS:
        for s in range(L):
            w_c = w0 + s
            sl = r * 26 + w_c if r < 3 else 78 + w_c
            hw = slots[sl]
            if hw is None:
                continue
            h, w = hw
            Wl = W[:, :, h, w, :]                 # [42, 3, 49]
            bl = b[:, h, w]                       # [42]
            for ci, t in enumerate(ts):
                col = (ck + ci) * 128 + 42 * s
                rel = 128 * t + np.arange(128) - STRIDE * w_c
                kj = rel // STRIDE
                jj = rel % STRIDE
                valid = (rel >= 0) & (rel < WINLEN) & (jj < 21)
                vals = np.zeros((128, OC), dtype=np.float32)
                vj, vk = jj[valid], kj[valid]
                vals[valid] = Wl[:, vj // 7, (vj % 7) * 7 + vk].T
                bias_row = (rel >= 0) & (rel < WINLEN) & (jj == 21) & (kj == 0)
                if bias_row.any():
                    vals[bias_row] = bl
                wt[:, col:col + OC] = vals.astype(ml_dtypes.bfloat16)
    # lwp ---------------------------------------------------------------
    lwp = np.zeros((128, NG * NCLS), dtype=ml_dtypes.bfloat16)
    for gi, (r, w0, L, ts, ck) in enumerate(GROUPS):
        for s in range(L):
            w_c = w0 + s
            sl = r * 26 + w_c if r < 3 else 78 + w_c
            if slots[sl] is None:
                continue
            h, w = slots[sl]
            lwp[42 * s:42 * s + OC, gi * NCLS:(gi + 1) * NCLS] = (
                lw4[:, :, h, w].T.astype(ml_dtypes.bfloat16)
            )
    return {"band": band, "wt": wt, "lwp": lwp}


def _run(x, W, b, lw, lb, trace=False):
    nc = _build_program()
    x = np.ascontiguousarray(np.asarray(x, dtype=np.float32))
    W = np.asarray(W, dtype=np.float32)
    b = np.asarray(b, dtype=np.float32)
    lw = np.asarray(lw, dtype=np.float32)
    lb = np.asarray(lb, dtype=np.float32)
    lw4 = lw.reshape(NCLS, OC, OH, OW)
    in_maps = [_prep_core(c, x, W, b, lw4) for c in range(NCORES)]
    res = run_bass_kernel_spmd(
        nc, in_maps, list(range(NCORES)), trace=trace,
    )
    part = np.zeros((4, NCLS, NB * NCHUNK), dtype=np.float32)
    for c in range(NCORES):
        part += res.results[c]["part"]
    out10 = part.sum(axis=0)
    out = out10.T + lb[None, :]
    return out.astype(np.float32), res


def kernel(**inputs):
    out, _ = _run(inputs["x"], inputs["W"], inputs["b"], inputs["lw"],
                  inputs["lb"])
    return out
